# revision 18
# baseline (speedup 1.0000x reference)
"""DiffusionGCN (2-layer GCN + linear head) on 8 Trainium2 NeuronCores.

Strategy (graph/data parallel):
  - Nodes sharded across 8 cores (12800 padded nodes each); edges partitioned
    by destination core, grouped by destination supertile (512 nodes) and
    source int16-range, and sorted by destination within each segment.
  - Symmetric-norm trick: out[v] = dis[v] * sum_{e: dst=v} (dis[src] * h[src]),
    so the source-side scale is folded into the gather table (htilde = dis*h)
    and the dest-side scale is applied after aggregation. W commutes with the
    aggregation and is applied after the segment-sum on the core's own shard.
  - Gathered source features are fetched with bulk `dma_gather` (SWDGE), one
    chunk per (supertile, range) segment. Per-segment padding lanes carry idx
    -1 at the tail of the chunk, which the SWDGE ucode trims; the per-core
    real count is supplied at runtime through a sequencer register, so
    descriptor generation (the measured bottleneck, ~2.4 ns/descriptor serial
    on the Q7 pair) only pays for real edges.
  - Segment-sum via windowed one-hot matmuls: edges are dst-sorted, so a
    128-edge block's destinations span a narrow window (~52 of 512 columns).
    dstloc is stored pre-shifted by each block's window base, so groups of 8
    blocks share one broadcast DVE is_equal against a 96-wide iota; each
    scatter matmul covers only its block's window columns. PSUM banks are
    initialized by a 1-partition zero outer-product matmul per supertile.
  - Feature-major epilogue: the aggregate is produced as agg[d, v], so
    z^T = W.(agg + dis*res^T) is two matmuls per 512-node supertile with the
    (constant) W.T as the stationary operand; bias is a per-partition column,
    dis multiplies along the free axis via a host-replicated [128, NV] table.
    Layer 1 emits h feature-major (fed straight to layer 2) plus the
    node-major dis*h gather table (4 PE transposes per supertile); layer 2
    fuses the classifier head as one matmul per supertile, storing logits
    [C, NV] which the host transposes.
  - 2 SPMD launches: layer 1, layer 2 + classifier head. Host computes deg ->
    dis and htilde0 = dis*x (cheap numpy), and re-shards between launches.
"""

import os
import sys
from contextlib import ExitStack

import numpy as np

for _p in ("/opt/trn_rl_repo", "/root/.axon_site/_ro/trn_rl_repo"):
    if os.path.isdir(_p) and _p not in sys.path:
        sys.path.insert(0, _p)

import ml_dtypes

import concourse.bacc as bacc
import concourse.bass as bass
import concourse.mybir as mybir
import concourse.tile as tile
from concourse.bass_utils import run_bass_kernel_spmd

F32 = mybir.dt.float32
F16 = mybir.dt.float16
BF16 = mybir.dt.bfloat16
I16 = mybir.dt.int16
I32 = mybir.dt.int32

N = 100000
E = 1600000
D = 128
H = 128
C = 64
NCORES = 8
NV = 12800               # nodes per core
NPAD = NV * NCORES       # 102400
NT = NV // 128           # 100 row-tiles per core
STW = 512                # supertile width (PSUM bank = 512 f32)
NSUP = NV // STW         # 25 supertiles per core
NRANGE = 4               # int16 gather-index ranges
RV = NPAD // NRANGE      # 25600 rows per range table
NSEG = NSUP * NRANGE

MMDT = BF16              # dtype of scatter-matmul operands
MSG_BUFS = 8
GRP = 8                  # blocks per one-hot is_equal op
WMAX = 96                # iota const width (>= any window width)


# ----------------------------------------------------------------- host prep

def _prep_graph(edge_index):
    """Partition/sort edges into the (supertile, range) segment grid with
    dst-sorted order inside each segment."""
    src_all = np.asarray(edge_index[0], dtype=np.int64)
    dst_all = np.asarray(edge_index[1], dtype=np.int64)

    # degree includes the self-loop; the self-loop itself is NOT in the edge
    # lists - its contribution (dis^2 * h_own) comes from the residual path in
    # the kernel epilogue.
    deg = (np.bincount(dst_all, minlength=NPAD)
           + np.concatenate([np.ones(N), np.zeros(NPAD - N)])).astype(np.float32)

    core = dst_all // NV
    sup = (dst_all % NV) // STW
    rr = src_all // RV

    counts = np.zeros((NCORES, NSUP, NRANGE), np.int64)
    np.add.at(counts, (core, sup, rr), 1)
    ks = np.maximum(np.ceil(counts.max(axis=0) / 128).astype(np.int64), 1)
    nb = int(ks.sum())
    flat_ks = ks.reshape(-1)
    gb_base = np.zeros(NSEG, np.int64)
    gb_base[1:] = np.cumsum(flat_ks)[:-1]
    gb_base = gb_base.reshape(NSUP, NRANGE)

    lane_src = np.full((NCORES, nb, 128), -1, np.int64)
    lane_dst = np.full((NCORES, nb, 128), -1.0, np.float32)
    kmax = int(ks.max())
    wlo = np.full((NSUP, NRANGE, kmax), 1 << 30, np.int64)
    whi = np.full((NSUP, NRANGE, kmax), -1, np.int64)
    for c in range(NCORES):
        m = core == c
        s_c = src_all[m]
        sup_c = sup[m]
        r_c = rr[m]
        dloc = (dst_all[m] % NV) - sup_c * STW
        # sort by (supertile, range, dst) - dst order keeps each block's
        # one-hot/matmul window narrow.
        order = np.lexsort((dloc, r_c, sup_c))
        s_c, sup_c, r_c, dloc = s_c[order], sup_c[order], r_c[order], dloc[order]
        loc = s_c - r_c * RV
        key = sup_c * NRANGE + r_c
        seg_starts = np.searchsorted(key, np.arange(NSEG))
        j = np.arange(len(s_c)) - seg_starts[key]
        blk = j // 128
        gb = gb_base[sup_c, r_c] + blk
        lane = j % 128
        lane_src[c, gb, lane] = loc
        lane_dst[c, gb, lane] = dloc
        np.minimum.at(wlo, (sup_c, r_c, blk), dloc)
        np.maximum.at(whi, (sup_c, r_c, blk), dloc)

    # per-block column windows (union over cores), even-aligned
    windows = []
    for s in range(NSUP):
        for r in range(NRANGE):
            for i in range(int(ks[s, r])):
                lo = int(wlo[s, r, i])
                hi = int(whi[s, r, i])
                if hi < 0:
                    lo, hi = 0, 1
                lo = max((lo // 2) * 2, 0)
                w = hi + 1 - lo
                w = min(((w + 1) // 2) * 2, STW - lo)
                windows.append((lo, w))
    assert len(windows) == nb
    # one-hot groups of GRP blocks share one is_equal op of width wg =
    # max block width in the group (rounded up); dstloc is stored shifted by
    # each block's window base so every block compares against iota[0:wg).
    wg = []
    for g0 in range(0, nb, GRP):
        mx = max(w for (_, w) in windows[g0:g0 + GRP])
        wg.append(((mx + 7) // 8) * 8)
    assert max(wg) <= WMAX

    # idx const [NC, 128, nb*8]: per-segment-chunk 16-lane wrap, replicated x8
    idx = np.zeros((NCORES, 128, nb * 8), np.int16)
    for s in range(NSUP):
        for r in range(NRANGE):
            k = int(ks[s, r])
            gb0 = int(gb_base[s, r])
            flat = lane_src[:, gb0:gb0 + k, :].reshape(NCORES, k * 128)
            wrapped = flat.reshape(NCORES, k * 8, 16).transpose(0, 2, 1)
            idx[:, :, gb0 * 8:(gb0 + k) * 8] = np.tile(
                wrapped, (1, 8, 1)).astype(np.int16)

    los = np.array([lo for (lo, _) in windows], np.float32)
    shifted = np.where(lane_dst < 0, -1.0, lane_dst - los[None, :, None])
    dstloc = shifted.transpose(0, 2, 1).astype(ml_dtypes.bfloat16)  # [NC,128,nb]
    cnts = counts.reshape(NCORES, NSEG).astype(np.int32)[:, None, :]
    return deg, ks, windows, wg, idx, dstloc, cnts


def _host_prep(x, edge_index):
    deg, ks, windows, wg, idx, dstloc, cnts = _prep_graph(edge_index)
    dis = np.where(deg > 0, 1.0 / np.sqrt(np.maximum(deg, 1.0)),
                   0.0).astype(np.float32)
    xp = _pad_rows(np.asarray(x, np.float32), NPAD)
    ht0 = (dis[:, None] * xp).astype(ml_dtypes.bfloat16)
    # dis laid out [128, NT] per core: dis_pc[c, p, t] = dis of node c*NV+t*128+p
    dis_pc = dis.reshape(NCORES, NT, 128).transpose(0, 2, 1).copy()
    # dis replicated across partitions, feature-major: disrep[c, p, v]=dis[c*NV+v]
    disrep = np.broadcast_to(
        dis.astype(np.float16).reshape(NCORES, 1, NV), (NCORES, 128, NV)).copy()
    # residual input for layer 1, feature-major
    xT = np.ascontiguousarray(xp.reshape(NCORES, NV, D).transpose(0, 2, 1))
    return ks, windows, wg, idx, dstloc, cnts, dis_pc, disrep, xT, ht0


# ------------------------------------------------------------ kernel builder

def _build_layer(ks, windows, wg, last):
    """One GCN layer. last=False: outputs h feature-major (next layer's res)
    and the node-major gather table htilde=dis*h. last=True: second layer
    fused with the classifier head, outputs logits [C, NV].

    Self-loop contribution is not gathered: z^T = W.(agg + dis*res^T) via a
    second accumulating matmul off the dis-scaled feature-major residual."""
    ks = np.asarray(ks, np.int64).reshape(NSUP, NRANGE)
    nb = int(ks.sum())
    kmax = int(ks.max())
    nc = bacc.Bacc("TRN2", num_swdge_queues=4)
    tables = [nc.dram_tensor(f"table{r}", [RV, D], MMDT, kind="ExternalInput")
              for r in range(NRANGE)]
    idxs = nc.dram_tensor("idxs", [128, nb * 8], I16, kind="ExternalInput")
    cnts_in = nc.dram_tensor("cnts", [1, NSEG], I32, kind="ExternalInput")
    dstl = nc.dram_tensor("dstl", [128, nb], BF16, kind="ExternalInput")
    res_in = nc.dram_tensor("res", [D, NV], F32, kind="ExternalInput")
    disrep_in = nc.dram_tensor("disrep", [128, NV], F16, kind="ExternalInput")
    wt_in = nc.dram_tensor("wt", [D, H], BF16, kind="ExternalInput")   # W.T
    bb_in = nc.dram_tensor("bb", [128, 1], F32, kind="ExternalInput")  # b column
    iota_in = nc.dram_tensor("iota", [128, WMAX], BF16, kind="ExternalInput")
    ones1_in = nc.dram_tensor("ones1", [1, 128], BF16, kind="ExternalInput")
    zrow_in = nc.dram_tensor("zrow", [1, STW], BF16, kind="ExternalInput")
    if last:
        wl_in = nc.dram_tensor("wl", [H, C], BF16, kind="ExternalInput")  # Wlin.T
        bl_in = nc.dram_tensor("bl", [C, 1], F32, kind="ExternalInput")
        out_lg = nc.dram_tensor("outlg", [C, NV], F32, kind="ExternalOutput")
    else:
        dis_in = nc.dram_tensor("dis", [128, NT], F32, kind="ExternalInput")
        id_in = nc.dram_tensor("ident", [128, 128], F32, kind="ExternalInput")
        out_h = nc.dram_tensor("outh", [D, NV], F32, kind="ExternalOutput")
        out_ht = nc.dram_tensor("outht", [NV, D], BF16, kind="ExternalOutput")

    with tile.TileContext(nc) as tc, ExitStack() as ctx:
        const = ctx.enter_context(tc.tile_pool(name="const", bufs=1))
        ohp = ctx.enter_context(tc.tile_pool(name="oh", bufs=8))
        aggp = ctx.enter_context(tc.tile_pool(name="agg", bufs=3))
        ep = ctx.enter_context(tc.tile_pool(name="ep", bufs=8))
        psum_st = ctx.enter_context(tc.tile_pool(name="pst", bufs=2, space="PSUM"))
        psum_z = ctx.enter_context(tc.tile_pool(name="pz", bufs=2, space="PSUM"))
        if last:
            psum_l = ctx.enter_context(tc.tile_pool(name="plg", bufs=2, space="PSUM"))
        else:
            psum_t = ctx.enter_context(tc.tile_pool(name="ptr", bufs=2, space="PSUM"))

        # small consts first so the first gather's dependencies land early;
        # the idx table is loaded in per-segment-aligned sections so gathers
        # start as soon as their own section arrives
        cnts_sb = const.tile([1, NSEG], I32)
        nc.sync.dma_start(cnts_sb[:], cnts_in[:])
        iota_sb = const.tile([128, WMAX], BF16)
        nc.sync.dma_start(iota_sb[:], iota_in[:])
        ones1_sb = const.tile([1, 128], BF16)
        nc.sync.dma_start(ones1_sb[:], ones1_in[:])
        zrow_sb = const.tile([1, STW], BF16)
        nc.sync.dma_start(zrow_sb[:], zrow_in[:])
        wt_sb = const.tile([D, H], BF16)
        nc.sync.dma_start(wt_sb[:], wt_in[:])
        bb_sb = const.tile([128, 1], F32)
        nc.sync.dma_start(bb_sb[:], bb_in[:])
        dstl_sb = const.tile([128, nb], BF16)
        nc.sync.dma_start(dstl_sb[:], dstl[:])
        idx_cut = min(nb, int(ks[0].sum())) * 8
        idx_a = const.tile([128, idx_cut], I16)
        nc.sync.dma_start(idx_a[:], idxs[:, :idx_cut])

        def idx_cols(c0, n):
            if c0 >= idx_cut:
                return idx_b[:, c0 - idx_cut:c0 - idx_cut + n]
            return idx_a[:, c0:c0 + n]
        # bulk constants are allocated here but their DMAs are issued only
        # after the first gather is in flight (see load_bulk below), keeping
        # the first gather's completion-semaphore waits off the big loads
        idx_b = const.tile([128, nb * 8 - idx_cut], I16)
        disrep_sb = const.tile([128, NV], F16)
        if last:
            wl_sb = const.tile([H, C], BF16)
            bl_sb = const.tile([C, 1], F32)
        else:
            dis_sb = const.tile([128, NT], F32)
            id_sb = const.tile([128, 128], F32)

        def load_bulk():
            nc.scalar.dma_start(idx_b[:], idxs[:, idx_cut:])
            nc.sync.dma_start(disrep_sb[:], disrep_in[:])
            if last:
                nc.sync.dma_start(wl_sb[:], wl_in[:])
                nc.sync.dma_start(bl_sb[:], bl_in[:])
            else:
                nc.sync.dma_start(dis_sb[:], dis_in[:])
                nc.sync.dma_start(id_sb[:], id_in[:])

        def epilogue(s, agg):
            sl = slice(s * STW, (s + 1) * STW)
            res_t = ep.tile([128, STW], F32)
            nc.sync.dma_start(res_t[:], res_in[:, sl])
            sresT = ep.tile([128, STW], BF16)
            nc.vector.tensor_tensor(out=sresT[:], in0=res_t[:],
                                    in1=disrep_sb[:, sl],
                                    op=mybir.AluOpType.mult)
            zT = psum_z.tile([128, STW], F32)
            nc.tensor.matmul(zT[:], lhsT=wt_sb[:], rhs=agg[:],
                             start=True, stop=False)
            nc.tensor.matmul(zT[:], lhsT=wt_sb[:], rhs=sresT[:],
                             start=False, stop=True)
            zs = ep.tile([128, STW], F32)
            nc.vector.tensor_tensor(out=zs[:], in0=zT[:],
                                    in1=disrep_sb[:, sl],
                                    op=mybir.AluOpType.mult)
            hr = ep.tile([128, STW], F32)
            nc.scalar.activation(hr[:], zs[:],
                                 mybir.ActivationFunctionType.Relu,
                                 bias=bb_sb[:, 0:1])
            h = ep.tile([128, STW], F32 if not last else BF16)
            nc.vector.tensor_tensor(out=h[:], in0=hr[:], in1=res_t[:],
                                    op=mybir.AluOpType.add)
            if not last:
                nc.scalar.dma_start(out_h[:, sl], h[:])
                for t2 in range(STW // 128):
                    t = (STW // 128) * s + t2
                    tp = psum_t.tile([128, 128], F32)
                    nc.tensor.transpose(tp[:], h[:, t2 * 128:(t2 + 1) * 128],
                                        id_sb[:])
                    ht = ep.tile([128, 128], BF16)
                    nc.scalar.activation(ht[:], tp[:],
                                         mybir.ActivationFunctionType.Identity,
                                         scale=dis_sb[:, t:t + 1])
                    nc.scalar.dma_start(out_ht[t * 128:(t + 1) * 128, :], ht[:])
            else:
                lgT = psum_l.tile([C, STW], F32)
                nc.tensor.matmul(lgT[:], lhsT=wl_sb[:], rhs=h[:],
                                 start=True, stop=True)
                lo = ep.tile([C, STW], F32)
                nc.scalar.activation(lo[:], lgT[:],
                                     mybir.ActivationFunctionType.Identity,
                                     bias=bl_sb[:, 0:1])
                nc.scalar.dma_start(out_lg[:, sl], lo[:])

        # persistent gather-destination ring: explicit rotation, fully
        # memset once upfront so lanes the trimmed gather skips always hold
        # finite stale data that the zero rows of the one-hot annihilate
        msgs = [const.tile([128, kmax * D], MMDT, name=f"msgbuf{i}")
                for i in range(MSG_BUFS)]
        for i, t in enumerate(msgs):
            if i % 2 == 0:
                nc.vector.memset(t[:], 0.0)
            else:
                nc.scalar.memzero(t[:])
        cnt_reg = nc.gpsimd.alloc_register("cntreg")

        def onehot(g0):
            # one is_equal for blocks [g0, g0+GRP): oh[:, j, :] is block
            # g0+j's one-hot over its shifted window [0, wgv)
            cnt = min(GRP, nb - g0)
            wgv = int(wg[g0 // GRP])
            oh = ohp.tile([128, GRP * WMAX], MMDT, tag="oh")
            dsl = dstl_sb[:, g0:g0 + cnt].to_broadcast([128, cnt, wgv])
            io_ap = iota_sb[:, :wgv]
            io_b = bass.AP(io_ap.tensor, io_ap.offset,
                           [io_ap.ap[0], [0, cnt], [1, wgv]])
            oh_view = oh[:, :cnt * wgv]
            oh3 = bass.AP(oh_view.tensor, oh_view.offset,
                          [oh_view.ap[0], [wgv, cnt], [1, wgv]])
            nc.vector.tensor_tensor(out=oh3, in0=dsl, in1=io_b,
                                    op=mybir.AluOpType.is_equal)
            return oh, wgv

        qctr = 0
        gb = 0
        col = 0
        cur_oh = (None, 0)
        for s in range(NSUP):
            ps = psum_st.tile([128, STW], F32)
            # init the accumulator: outer product of a zero row
            nc.tensor.matmul(ps[:], lhsT=ones1_sb[:], rhs=zrow_sb[:],
                             start=True, stop=False)
            for r in range(NRANGE):
                k = int(ks[s, r])
                t = msgs[qctr % MSG_BUFS]
                m = t[:]
                out3 = bass.AP(m.tensor, m.offset,
                               [m.ap[0], [D, k], [1, D]])
                lanes = k * 128
                nc.gpsimd.reg_load(cnt_reg, cnts_sb[0:1, s * NRANGE + r:s * NRANGE + r + 1])
                nc.gpsimd.dma_gather(
                    out3, tables[r][:, :],
                    idx_cols(col, k * 8),
                    lanes, cnt_reg, D, single_packet=False,
                    queue_num=qctr % 4)
                if qctr == 0:
                    load_bulk()
                qctr += 1
                for i in range(k):
                    if gb % GRP == 0:
                        cur_oh = onehot(gb)
                    oh, wgv = cur_oh
                    lo_w, w = windows[gb]
                    j = gb % GRP
                    is_last = (r == NRANGE - 1) and (i == k - 1)
                    nc.tensor.matmul(ps[:, lo_w:lo_w + w],
                                     lhsT=t[:, i * D:(i + 1) * D],
                                     rhs=oh[:, j * wgv:j * wgv + w],
                                     start=False, stop=is_last)
                    gb += 1
                col += k * 8
            agg = aggp.tile([128, STW], BF16)
            nc.scalar.copy(agg[:], ps[:])
            epilogue(s, agg)
    nc.finalize()
    return nc


# ------------------------------------------------------------------- driver

def _pad_rows(a, rows):
    out = np.zeros((rows, a.shape[1]), dtype=a.dtype)
    out[: a.shape[0]] = a
    return out


_cache = {}


def _consts(last):
    cn = dict(
        iota=np.tile(np.arange(WMAX, dtype=ml_dtypes.bfloat16), (128, 1)),
        ones1=np.ones((1, 128), dtype=ml_dtypes.bfloat16),
        zrow=np.zeros((1, STW), dtype=ml_dtypes.bfloat16))
    if not last:
        cn["ident"] = np.eye(128, dtype=np.float32)
    return cn


def _tabs(ht):
    return {f"table{r}": ht[r * RV:(r + 1) * RV] for r in range(NRANGE)}


def kernel(x, edge_index, W1, b1, W2, b2, Wlin, blin):
    x = np.asarray(x, dtype=np.float32)
    W1 = np.asarray(W1, dtype=np.float32)
    b1 = np.asarray(b1, dtype=np.float32)
    W2 = np.asarray(W2, dtype=np.float32)
    b2 = np.asarray(b2, dtype=np.float32)
    Wlin = np.asarray(Wlin, dtype=np.float32)
    blin = np.asarray(blin, dtype=np.float32)

    (ks, windows, wg, idx, dstloc, cnts, dis_pc, disrep, xT,
     ht0) = _host_prep(x, edge_index)
    cores = list(range(NCORES))

    key = (tuple(int(k) for k in ks.reshape(-1)), tuple(windows), tuple(wg))
    if _cache.get("key") != key:
        _cache.clear()
        _cache["key"] = key
        _cache["l1"] = _build_layer(ks, windows, wg, last=False)
        _cache["l2"] = _build_layer(ks, windows, wg, last=True)

    # ---- launch 1: layer 1
    in1 = [{**_tabs(ht0), "idxs": idx[c], "cnts": cnts[c], "dstl": dstloc[c],
            "res": xT[c], "dis": dis_pc[c], "disrep": disrep[c],
            "wt": W1.T.astype(ml_dtypes.bfloat16),
            "bb": b1[:, None], **_consts(last=False)}
           for c in cores]
    r1 = run_bass_kernel_spmd(_cache["l1"], in1, cores)
    h1_pc = [r1.results[c]["outh"] for c in cores]
    ht1 = np.concatenate([r1.results[c]["outht"] for c in cores])

    # ---- launch 2: layer 2 + head
    in2 = [{**_tabs(ht1), "idxs": idx[c], "cnts": cnts[c], "dstl": dstloc[c],
            "res": h1_pc[c], "disrep": disrep[c],
            "wt": W2.T.astype(ml_dtypes.bfloat16),
            "bb": b2[:, None],
            "wl": Wlin.T.astype(ml_dtypes.bfloat16),
            "bl": blin[:, None], **_consts(last=True)} for c in cores]
    r2 = run_bass_kernel_spmd(_cache["l2"], in2, cores)
    logits = np.concatenate([r2.results[c]["outlg"].T for c in cores])
    return logits[:N].astype(np.float32)


# revision 19
# speedup vs baseline: 1.0305x; 1.0305x over previous
"""DiffusionGCN (2-layer GCN + linear head) on 8 Trainium2 NeuronCores.

Strategy (graph/data parallel):
  - Nodes sharded across 8 cores (12800 padded nodes each); edges partitioned
    by destination core, grouped by destination supertile (512 nodes) and
    source int16-range, and sorted by destination within each segment.
  - Symmetric-norm trick: out[v] = dis[v] * sum_{e: dst=v} (dis[src] * h[src]),
    so the source-side scale is folded into the gather table (htilde = dis*h)
    and the dest-side scale is applied after aggregation. W commutes with the
    aggregation and is applied after the segment-sum on the core's own shard.
  - Gathered source features are fetched with bulk `dma_gather` (SWDGE), one
    chunk per (supertile, range) segment. Per-segment padding lanes carry idx
    -1 at the tail of the chunk, which the SWDGE ucode trims; the per-core
    real count is supplied at runtime through a sequencer register, so
    descriptor generation (the measured bottleneck, ~2.4 ns/descriptor serial
    on the Q7 pair) only pays for real edges.
  - Segment-sum via windowed one-hot matmuls: edges are dst-sorted, so a
    128-edge block's destinations span a narrow window (~52 of 512 columns).
    dstloc is stored pre-shifted by each block's window base, so groups of 8
    blocks share one broadcast DVE is_equal against a 96-wide iota; each
    scatter matmul covers only its block's window columns. PSUM banks are
    initialized by a 1-partition zero outer-product matmul per supertile.
  - Feature-major epilogue: the aggregate is produced as agg[d, v], so
    z^T = W.(agg + dis*res^T) is two matmuls per 512-node supertile with the
    (constant) W.T as the stationary operand; bias is a per-partition column,
    dis multiplies along the free axis via a host-replicated [128, NV] table.
    Layer 1 emits h feature-major (fed straight to layer 2) plus the
    node-major dis*h gather table (4 PE transposes per supertile); layer 2
    fuses the classifier head as one matmul per supertile, storing logits
    [C, NV] which the host transposes.
  - 2 SPMD launches: layer 1, layer 2 + classifier head. Host computes deg ->
    dis and htilde0 = dis*x (cheap numpy), and re-shards between launches.
"""

import os
import sys
from contextlib import ExitStack

import numpy as np

for _p in ("/opt/trn_rl_repo", "/root/.axon_site/_ro/trn_rl_repo"):
    if os.path.isdir(_p) and _p not in sys.path:
        sys.path.insert(0, _p)

import ml_dtypes

import concourse.bacc as bacc
import concourse.bass as bass
from concourse import library_config
import concourse.mybir as mybir
import concourse.tile as tile
from concourse.bass_utils import run_bass_kernel_spmd

F32 = mybir.dt.float32
F16 = mybir.dt.float16
BF16 = mybir.dt.bfloat16
I16 = mybir.dt.int16
I32 = mybir.dt.int32

N = 100000
E = 1600000
D = 128
H = 128
C = 64
NCORES = 8
NV = 12800               # nodes per core
NPAD = NV * NCORES       # 102400
NT = NV // 128           # 100 row-tiles per core
STW = 512                # supertile width (PSUM bank = 512 f32)
NSUP = NV // STW         # 25 supertiles per core
NRANGE = 4               # int16 gather-index ranges
RV = NPAD // NRANGE      # 25600 rows per range table
NSEG = NSUP * NRANGE

MMDT = BF16              # dtype of scatter-matmul operands
MSG_BUFS = 8
GRP = 8                  # blocks per one-hot is_equal op
WMAX = 96                # iota const width (>= any window width)


# ----------------------------------------------------------------- host prep

def _prep_graph(edge_index):
    """Partition/sort edges into the (supertile, range) segment grid with
    dst-sorted order inside each segment."""
    src_all = np.asarray(edge_index[0], dtype=np.int64)
    dst_all = np.asarray(edge_index[1], dtype=np.int64)

    # degree includes the self-loop; the self-loop itself is NOT in the edge
    # lists - its contribution (dis^2 * h_own) comes from the residual path in
    # the kernel epilogue.
    deg = (np.bincount(dst_all, minlength=NPAD)
           + np.concatenate([np.ones(N), np.zeros(NPAD - N)])).astype(np.float32)

    core = dst_all // NV
    sup = (dst_all % NV) // STW
    rr = src_all // RV

    counts = np.zeros((NCORES, NSUP, NRANGE), np.int64)
    np.add.at(counts, (core, sup, rr), 1)
    ks = np.maximum(np.ceil(counts.max(axis=0) / 128).astype(np.int64), 1)
    nb = int(ks.sum())
    flat_ks = ks.reshape(-1)
    gb_base = np.zeros(NSEG, np.int64)
    gb_base[1:] = np.cumsum(flat_ks)[:-1]
    gb_base = gb_base.reshape(NSUP, NRANGE)

    lane_src = np.full((NCORES, nb, 128), -1, np.int64)
    lane_dst = np.full((NCORES, nb, 128), -1.0, np.float32)
    kmax = int(ks.max())
    wlo = np.full((NSUP, NRANGE, kmax), 1 << 30, np.int64)
    whi = np.full((NSUP, NRANGE, kmax), -1, np.int64)
    for c in range(NCORES):
        m = core == c
        s_c = src_all[m]
        sup_c = sup[m]
        r_c = rr[m]
        dloc = (dst_all[m] % NV) - sup_c * STW
        # sort by (supertile, range, dst) - dst order keeps each block's
        # one-hot/matmul window narrow.
        order = np.lexsort((dloc, r_c, sup_c))
        s_c, sup_c, r_c, dloc = s_c[order], sup_c[order], r_c[order], dloc[order]
        loc = s_c - r_c * RV
        key = sup_c * NRANGE + r_c
        seg_starts = np.searchsorted(key, np.arange(NSEG))
        j = np.arange(len(s_c)) - seg_starts[key]
        blk = j // 128
        gb = gb_base[sup_c, r_c] + blk
        lane = j % 128
        lane_src[c, gb, lane] = loc
        lane_dst[c, gb, lane] = dloc
        np.minimum.at(wlo, (sup_c, r_c, blk), dloc)
        np.maximum.at(whi, (sup_c, r_c, blk), dloc)

    # per-block column windows (union over cores), even-aligned
    windows = []
    for s in range(NSUP):
        for r in range(NRANGE):
            for i in range(int(ks[s, r])):
                lo = int(wlo[s, r, i])
                hi = int(whi[s, r, i])
                if hi < 0:
                    lo, hi = 0, 1
                lo = max((lo // 2) * 2, 0)
                w = hi + 1 - lo
                w = min(((w + 1) // 2) * 2, STW - lo)
                windows.append((lo, w))
    assert len(windows) == nb
    # one-hot groups of GRP blocks share one is_equal op of width wg =
    # max block width in the group (rounded up); dstloc is stored shifted by
    # each block's window base so every block compares against iota[0:wg).
    wg = []
    for g0 in range(0, nb, GRP):
        mx = max(w for (_, w) in windows[g0:g0 + GRP])
        wg.append(((mx + 7) // 8) * 8)
    assert max(wg) <= WMAX

    # idx const [NC, 128, nb*8]: per-segment-chunk 16-lane wrap, replicated x8
    idx = np.zeros((NCORES, 128, nb * 8), np.int16)
    for s in range(NSUP):
        for r in range(NRANGE):
            k = int(ks[s, r])
            gb0 = int(gb_base[s, r])
            flat = lane_src[:, gb0:gb0 + k, :].reshape(NCORES, k * 128)
            wrapped = flat.reshape(NCORES, k * 8, 16).transpose(0, 2, 1)
            idx[:, :, gb0 * 8:(gb0 + k) * 8] = np.tile(
                wrapped, (1, 8, 1)).astype(np.int16)

    los = np.array([lo for (lo, _) in windows], np.float32)
    shifted = np.where(lane_dst < 0, -1.0, lane_dst - los[None, :, None])
    dstloc = shifted.transpose(0, 2, 1).astype(ml_dtypes.bfloat16)  # [NC,128,nb]
    cnts = counts.reshape(NCORES, NSEG).astype(np.int32)[:, None, :]
    return deg, ks, windows, wg, idx, dstloc, cnts


def _host_prep(x, edge_index):
    deg, ks, windows, wg, idx, dstloc, cnts = _prep_graph(edge_index)
    dis = np.where(deg > 0, 1.0 / np.sqrt(np.maximum(deg, 1.0)),
                   0.0).astype(np.float32)
    xp = _pad_rows(np.asarray(x, np.float32), NPAD)
    ht0 = (dis[:, None] * xp).astype(ml_dtypes.bfloat16)
    # dis laid out [128, NT] per core: dis_pc[c, p, t] = dis of node c*NV+t*128+p
    dis_pc = dis.reshape(NCORES, NT, 128).transpose(0, 2, 1).copy()
    # dis replicated across partitions, feature-major: disrep[c, p, v]=dis[c*NV+v]
    disrep = np.broadcast_to(
        dis.astype(np.float16).reshape(NCORES, 1, NV), (NCORES, 128, NV)).copy()
    # residual input for layer 1, feature-major
    xT = np.ascontiguousarray(xp.reshape(NCORES, NV, D).transpose(0, 2, 1))
    return ks, windows, wg, idx, dstloc, cnts, dis_pc, disrep, xT, ht0


# ------------------------------------------------------------ kernel builder

def _build_layer(ks, windows, wg, last):
    """One GCN layer. last=False: outputs h feature-major (next layer's res)
    and the node-major gather table htilde=dis*h. last=True: second layer
    fused with the classifier head, outputs logits [C, NV].

    Self-loop contribution is not gathered: z^T = W.(agg + dis*res^T) via a
    second accumulating matmul off the dis-scaled feature-major residual."""
    ks = np.asarray(ks, np.int64).reshape(NSUP, NRANGE)
    nb = int(ks.sum())
    kmax = int(ks.max())
    nc = bacc.Bacc("TRN2", num_swdge_queues=4)
    tables = [nc.dram_tensor(f"table{r}", [RV, D], MMDT, kind="ExternalInput")
              for r in range(NRANGE)]
    idxs = nc.dram_tensor("idxs", [128, nb * 8], I16, kind="ExternalInput")
    cnts_in = nc.dram_tensor("cnts", [1, NSEG], I32, kind="ExternalInput")
    dstl = nc.dram_tensor("dstl", [128, nb], BF16, kind="ExternalInput")
    res_in = nc.dram_tensor("res", [D, NV], F32, kind="ExternalInput")
    disrep_in = nc.dram_tensor("disrep", [128, NV], F16, kind="ExternalInput")
    wt_in = nc.dram_tensor("wt", [D, H], BF16, kind="ExternalInput")   # W.T
    bb_in = nc.dram_tensor("bb", [128, 1], F32, kind="ExternalInput")  # b column
    iota_in = nc.dram_tensor("iota", [128, WMAX], BF16, kind="ExternalInput")
    ones1_in = nc.dram_tensor("ones1", [1, 128], BF16, kind="ExternalInput")
    zrow_in = nc.dram_tensor("zrow", [1, STW], BF16, kind="ExternalInput")
    if last:
        wl_in = nc.dram_tensor("wl", [H, C], BF16, kind="ExternalInput")  # Wlin.T
        bl_in = nc.dram_tensor("bl", [C, 1], F32, kind="ExternalInput")
        out_lg = nc.dram_tensor("outlg", [C, NV], F32, kind="ExternalOutput")
    else:
        dis_in = nc.dram_tensor("dis", [128, NT], F32, kind="ExternalInput")
        id_in = nc.dram_tensor("ident", [128, 128], F32, kind="ExternalInput")
        out_h = nc.dram_tensor("outh", [D, NV], F32, kind="ExternalOutput")
        out_ht = nc.dram_tensor("outht", [NV, D], BF16, kind="ExternalOutput")

    with tile.TileContext(nc) as tc, ExitStack() as ctx:
        # preload the Q7 library holding dma_gather so its ~6us IRAM load
        # overlaps the constant DMAs instead of gating the first gather
        nc.gpsimd.load_library(library_config.mlp)
        const = ctx.enter_context(tc.tile_pool(name="const", bufs=1))
        ohp = ctx.enter_context(tc.tile_pool(name="oh", bufs=8))
        aggp = ctx.enter_context(tc.tile_pool(name="agg", bufs=3))
        ep = ctx.enter_context(tc.tile_pool(name="ep", bufs=8))
        psum_st = ctx.enter_context(tc.tile_pool(name="pst", bufs=2, space="PSUM"))
        psum_z = ctx.enter_context(tc.tile_pool(name="pz", bufs=2, space="PSUM"))
        if last:
            psum_l = ctx.enter_context(tc.tile_pool(name="plg", bufs=2, space="PSUM"))
        else:
            psum_t = ctx.enter_context(tc.tile_pool(name="ptr", bufs=2, space="PSUM"))

        # small consts first so the first gather's dependencies land early;
        # the idx table is loaded in per-segment-aligned sections so gathers
        # start as soon as their own section arrives
        cnts_sb = const.tile([1, NSEG], I32)
        nc.sync.dma_start(cnts_sb[:], cnts_in[:])
        iota_sb = const.tile([128, WMAX], BF16)
        nc.sync.dma_start(iota_sb[:], iota_in[:])
        ones1_sb = const.tile([1, 128], BF16)
        nc.sync.dma_start(ones1_sb[:], ones1_in[:])
        zrow_sb = const.tile([1, STW], BF16)
        nc.sync.dma_start(zrow_sb[:], zrow_in[:])
        wt_sb = const.tile([D, H], BF16)
        nc.sync.dma_start(wt_sb[:], wt_in[:])
        bb_sb = const.tile([128, 1], F32)
        nc.sync.dma_start(bb_sb[:], bb_in[:])
        dstl_sb = const.tile([128, nb], BF16)
        nc.sync.dma_start(dstl_sb[:], dstl[:])
        idx_cut = min(nb, int(ks[0].sum())) * 8
        idx_a = const.tile([128, idx_cut], I16)
        nc.sync.dma_start(idx_a[:], idxs[:, :idx_cut])

        def idx_cols(c0, n):
            if c0 >= idx_cut:
                return idx_b[:, c0 - idx_cut:c0 - idx_cut + n]
            return idx_a[:, c0:c0 + n]
        # bulk constants are allocated here but their DMAs are issued only
        # after the first gather is in flight (see load_bulk below), keeping
        # the first gather's completion-semaphore waits off the big loads
        idx_b = const.tile([128, nb * 8 - idx_cut], I16)
        disrep_sb = const.tile([128, NV], F16)
        if last:
            wl_sb = const.tile([H, C], BF16)
            bl_sb = const.tile([C, 1], F32)
        else:
            dis_sb = const.tile([128, NT], F32)
            id_sb = const.tile([128, 128], F32)

        def load_bulk():
            nc.scalar.dma_start(idx_b[:], idxs[:, idx_cut:])
            nc.sync.dma_start(disrep_sb[:], disrep_in[:])
            if last:
                nc.sync.dma_start(wl_sb[:], wl_in[:])
                nc.sync.dma_start(bl_sb[:], bl_in[:])
            else:
                nc.sync.dma_start(dis_sb[:], dis_in[:])
                nc.sync.dma_start(id_sb[:], id_in[:])

        def epilogue(s, agg):
            sl = slice(s * STW, (s + 1) * STW)
            res_t = ep.tile([128, STW], F32)
            nc.sync.dma_start(res_t[:], res_in[:, sl])
            sresT = ep.tile([128, STW], BF16)
            nc.vector.tensor_tensor(out=sresT[:], in0=res_t[:],
                                    in1=disrep_sb[:, sl],
                                    op=mybir.AluOpType.mult)
            zT = psum_z.tile([128, STW], F32)
            nc.tensor.matmul(zT[:], lhsT=wt_sb[:], rhs=agg[:],
                             start=True, stop=False)
            nc.tensor.matmul(zT[:], lhsT=wt_sb[:], rhs=sresT[:],
                             start=False, stop=True)
            zs = ep.tile([128, STW], F32)
            nc.vector.tensor_tensor(out=zs[:], in0=zT[:],
                                    in1=disrep_sb[:, sl],
                                    op=mybir.AluOpType.mult)
            hr = ep.tile([128, STW], F32)
            nc.scalar.activation(hr[:], zs[:],
                                 mybir.ActivationFunctionType.Relu,
                                 bias=bb_sb[:, 0:1])
            h = ep.tile([128, STW], F32 if not last else BF16)
            nc.vector.tensor_tensor(out=h[:], in0=hr[:], in1=res_t[:],
                                    op=mybir.AluOpType.add)
            if not last:
                nc.scalar.dma_start(out_h[:, sl], h[:])
                for t2 in range(STW // 128):
                    t = (STW // 128) * s + t2
                    tp = psum_t.tile([128, 128], F32)
                    nc.tensor.transpose(tp[:], h[:, t2 * 128:(t2 + 1) * 128],
                                        id_sb[:])
                    ht = ep.tile([128, 128], BF16)
                    nc.scalar.activation(ht[:], tp[:],
                                         mybir.ActivationFunctionType.Identity,
                                         scale=dis_sb[:, t:t + 1])
                    nc.scalar.dma_start(out_ht[t * 128:(t + 1) * 128, :], ht[:])
            else:
                lgT = psum_l.tile([C, STW], F32)
                nc.tensor.matmul(lgT[:], lhsT=wl_sb[:], rhs=h[:],
                                 start=True, stop=True)
                lo = ep.tile([C, STW], F32)
                nc.scalar.activation(lo[:], lgT[:],
                                     mybir.ActivationFunctionType.Identity,
                                     bias=bl_sb[:, 0:1])
                nc.scalar.dma_start(out_lg[:, sl], lo[:])

        # persistent gather-destination ring: explicit rotation, fully
        # memset once upfront so lanes the trimmed gather skips always hold
        # finite stale data that the zero rows of the one-hot annihilate
        msgs = [const.tile([128, kmax * D], MMDT, name=f"msgbuf{i}")
                for i in range(MSG_BUFS)]
        for i, t in enumerate(msgs):
            if i % 2 == 0:
                nc.vector.memset(t[:], 0.0)
            else:
                nc.scalar.memzero(t[:])
        cnt_reg = nc.gpsimd.alloc_register("cntreg")

        def onehot(g0):
            # one is_equal for blocks [g0, g0+GRP): oh[:, j, :] is block
            # g0+j's one-hot over its shifted window [0, wgv)
            cnt = min(GRP, nb - g0)
            wgv = int(wg[g0 // GRP])
            oh = ohp.tile([128, GRP * WMAX], MMDT, tag="oh")
            dsl = dstl_sb[:, g0:g0 + cnt].to_broadcast([128, cnt, wgv])
            io_ap = iota_sb[:, :wgv]
            io_b = bass.AP(io_ap.tensor, io_ap.offset,
                           [io_ap.ap[0], [0, cnt], [1, wgv]])
            oh_view = oh[:, :cnt * wgv]
            oh3 = bass.AP(oh_view.tensor, oh_view.offset,
                          [oh_view.ap[0], [wgv, cnt], [1, wgv]])
            nc.vector.tensor_tensor(out=oh3, in0=dsl, in1=io_b,
                                    op=mybir.AluOpType.is_equal)
            return oh, wgv

        qctr = 0
        gb = 0
        col = 0
        cur_oh = (None, 0)
        for s in range(NSUP):
            ps = psum_st.tile([128, STW], F32)
            # init the accumulator: outer product of a zero row
            nc.tensor.matmul(ps[:], lhsT=ones1_sb[:], rhs=zrow_sb[:],
                             start=True, stop=False)
            for r in range(NRANGE):
                k = int(ks[s, r])
                t = msgs[qctr % MSG_BUFS]
                m = t[:]
                out3 = bass.AP(m.tensor, m.offset,
                               [m.ap[0], [D, k], [1, D]])
                lanes = k * 128
                nc.gpsimd.reg_load(cnt_reg, cnts_sb[0:1, s * NRANGE + r:s * NRANGE + r + 1])
                nc.gpsimd.dma_gather(
                    out3, tables[r][:, :],
                    idx_cols(col, k * 8),
                    lanes, cnt_reg, D, single_packet=False,
                    queue_num=qctr % 4)
                if qctr == 0:
                    load_bulk()
                qctr += 1
                for i in range(k):
                    if gb % GRP == 0:
                        cur_oh = onehot(gb)
                    oh, wgv = cur_oh
                    lo_w, w = windows[gb]
                    j = gb % GRP
                    is_last = (r == NRANGE - 1) and (i == k - 1)
                    nc.tensor.matmul(ps[:, lo_w:lo_w + w],
                                     lhsT=t[:, i * D:(i + 1) * D],
                                     rhs=oh[:, j * wgv:j * wgv + w],
                                     start=False, stop=is_last)
                    gb += 1
                col += k * 8
            agg = aggp.tile([128, STW], BF16)
            nc.scalar.copy(agg[:], ps[:])
            epilogue(s, agg)
    nc.finalize()
    return nc


# ------------------------------------------------------------------- driver

def _pad_rows(a, rows):
    out = np.zeros((rows, a.shape[1]), dtype=a.dtype)
    out[: a.shape[0]] = a
    return out


_cache = {}


def _consts(last):
    cn = dict(
        iota=np.tile(np.arange(WMAX, dtype=ml_dtypes.bfloat16), (128, 1)),
        ones1=np.ones((1, 128), dtype=ml_dtypes.bfloat16),
        zrow=np.zeros((1, STW), dtype=ml_dtypes.bfloat16))
    if not last:
        cn["ident"] = np.eye(128, dtype=np.float32)
    return cn


def _tabs(ht):
    return {f"table{r}": ht[r * RV:(r + 1) * RV] for r in range(NRANGE)}


def kernel(x, edge_index, W1, b1, W2, b2, Wlin, blin):
    x = np.asarray(x, dtype=np.float32)
    W1 = np.asarray(W1, dtype=np.float32)
    b1 = np.asarray(b1, dtype=np.float32)
    W2 = np.asarray(W2, dtype=np.float32)
    b2 = np.asarray(b2, dtype=np.float32)
    Wlin = np.asarray(Wlin, dtype=np.float32)
    blin = np.asarray(blin, dtype=np.float32)

    (ks, windows, wg, idx, dstloc, cnts, dis_pc, disrep, xT,
     ht0) = _host_prep(x, edge_index)
    cores = list(range(NCORES))

    key = (tuple(int(k) for k in ks.reshape(-1)), tuple(windows), tuple(wg))
    if _cache.get("key") != key:
        _cache.clear()
        _cache["key"] = key
        _cache["l1"] = _build_layer(ks, windows, wg, last=False)
        _cache["l2"] = _build_layer(ks, windows, wg, last=True)

    # ---- launch 1: layer 1
    in1 = [{**_tabs(ht0), "idxs": idx[c], "cnts": cnts[c], "dstl": dstloc[c],
            "res": xT[c], "dis": dis_pc[c], "disrep": disrep[c],
            "wt": W1.T.astype(ml_dtypes.bfloat16),
            "bb": b1[:, None], **_consts(last=False)}
           for c in cores]
    r1 = run_bass_kernel_spmd(_cache["l1"], in1, cores)
    h1_pc = [r1.results[c]["outh"] for c in cores]
    ht1 = np.concatenate([r1.results[c]["outht"] for c in cores])

    # ---- launch 2: layer 2 + head
    in2 = [{**_tabs(ht1), "idxs": idx[c], "cnts": cnts[c], "dstl": dstloc[c],
            "res": h1_pc[c], "disrep": disrep[c],
            "wt": W2.T.astype(ml_dtypes.bfloat16),
            "bb": b2[:, None],
            "wl": Wlin.T.astype(ml_dtypes.bfloat16),
            "bl": blin[:, None], **_consts(last=True)} for c in cores]
    r2 = run_bass_kernel_spmd(_cache["l2"], in2, cores)
    logits = np.concatenate([r2.results[c]["outlg"].T for c in cores])
    return logits[:N].astype(np.float32)
